# revision 46
# baseline (speedup 1.0000x reference)
"""Trainium2 Bass kernel for nn_AdaAug (scatter_memory).

Computation (per sample i, kriged node k):
    r          = offs[i] + krig_idx[i,k]            # flat row index
    smp        = y[r, :]                            # gather
    h          = relu(smp @ W1 + b1)
    logits     = h @ W2 + b2
    ind        = argmax(logits + gumbel) == 1       # hard gumbel-softmax fwd
    out        = x, with out[r, :] = ind * mask * smp

Sharding: data-parallel over batch across 8 NeuronCores (64 samples per
core); MLP weights replicated; gathers/scatters are device-local because
krig_idx offsets stay within each sample's 500-row block.

v6 design notes (fp16 datapath, multi-queue SWDGE, padded token stream):
  - fp16 datapath end-to-end (measured rel err 2.9e-3 vs the 2e-2 gate;
    2 indicator flips on the reference inputs).  The d (logit-diff)
    reduction stays fp32 on f32 hidden activations.
  - The 6400 kriged tokens per core are re-ordered host-side into 8
    sample-aligned units of 800 tokens, each padded to 896 (7 blocks)
    with dummy tokens.  Scatter piece u targets its own 4000-row output
    tensor (disjoint rows, so Tile sees no write-after-write hazard and
    the 8 scatter-adds on SWDGE queues 0-3 run concurrently); its 96 pad
    indices are -1, which the ucode legally drops.  Gathers use the same
    stream.  Concurrent SWDGE queues each use one Q7 pair, so descriptor
    generation for gathers and scatters is ~4x one queue.
  - MLP: per 4-block batch, 4 fp16 PE transposes -> one DVE PSUM->SBUF
    copy -> one fp16 W1 matmul (b1 folded in via a constant-1 pad column
    of y and a b1 row of W1) -> ScalarE relu into 4 partition bands
    ("d-fold") -> one fp32 matmul against block-diagonal w2d [128,4]
    gives d for 4 blocks; a [4,128]->[128,4] PE transpose lands d
    token-major; DVE is_gt against host-precomputed -(gumbel_diff +
    b2_diff) gives the indicator.  (Transpose-gather is NOT used:
    concurrent transpose-gathers corrupt each other in the shared xbar.)
  - bulk x->out copies (fp16, one per 4000-row unit) issued first;
    kriged rows pre-zeroed host-side so scatter-add adds onto zeros.
"""

import sys

import numpy as np

for _p in ("/opt/trn_rl_repo", "/opt/pypackages"):
    if _p not in sys.path:
        sys.path.insert(0, _p)

M = 8                 # cores
BS, N, K, S = 512, 500, 100, 96
HID, AUG = 32, 2
B = BS // M           # samples per core
R = B * N             # x/y rows per core
J = B * K             # real kriged rows per core
P = 128               # SBUF partitions
SPH = 128             # padded fp16 row width (256B)
U = 8                 # scatter units per core
SAMP_U = B // U       # samples per unit (8)
ROWS_U = SAMP_U * N   # x rows per unit (4000)
JU = SAMP_U * K       # real tokens per unit (800)
BU = 7                # blocks per unit (896 padded tokens)
ROWS_O = ROWS_U + 4   # out tensor rows incl. sacrificial pad-target rows
JP = U * BU * P       # padded tokens per core (7168)
TP = JP // P          # padded blocks (56)

PIECES = (BU,) * U                      # gather/MLP pieces == scatter units
GQ = (0, 1, 2, 3, 0, 1, 2, 3)
NG = len(PIECES)
POFF = [sum(PIECES[:i]) for i in range(NG + 1)]
SQ = (1, 2, 3, 0, 1, 2, 3, 0)           # scatter unit -> queue

# f32 const blob columns: [ident4(4) | w2d_bd(4) | ne(TP)]
FB_W2D = 4
FB_NE = 8
FB_COLS = FB_NE + TP

_cache = {}


def _build():
    from contextlib import ExitStack

    import concourse.tile as tile
    from concourse import bacc, library_config, mybir

    f32 = mybir.dt.float32
    f16 = mybir.dt.float16
    i16 = mybir.dt.int16

    nc = bacc.Bacc(
        "TRN2",
        target_bir_lowering=False,
        debug=False,
        num_devices=M,
        num_swdge_queues=4,
    )

    x_e = nc.dram_tensor("x", [R, S], f16, kind="ExternalInput")
    yq_e = nc.dram_tensor("yq", [R, SPH], f16, kind="ExternalInput")
    mask_e = nc.dram_tensor("mask", [P, TP * S], f16, kind="ExternalInput")
    ib_e = nc.dram_tensor("ib", [P, 2 * (JP // 16)], i16, kind="ExternalInput")
    fb_e = nc.dram_tensor("fb", [P, FB_COLS], f32, kind="ExternalInput")
    wb_e = nc.dram_tensor("wb", [P, HID + P], f16, kind="ExternalInput")
    outs = [
        nc.dram_tensor(f"out{u}", [ROWS_O, SPH], f16, kind="ExternalOutput")
        for u in range(U)
    ]

    with tile.TileContext(nc) as tc, ExitStack() as ctx:
        const = ctx.enter_context(tc.tile_pool(name="const", bufs=1))
        big = ctx.enter_context(tc.tile_pool(name="big", bufs=1))
        work = ctx.enter_context(tc.tile_pool(name="work", bufs=6))
        pp = ctx.enter_context(tc.tile_pool(name="pp", bufs=3, space="PSUM"))
        pph = ctx.enter_context(tc.tile_pool(name="pph", bufs=2, space="PSUM"))
        pp4 = ctx.enter_context(tc.tile_pool(name="pp4", bufs=2, space="PSUM"))
        ppt = ctx.enter_context(tc.tile_pool(name="ppt", bufs=1, space="PSUM"))

        nc.gpsimd.load_library(library_config.mlp)

        # Gather-index blob at the head of the quiet Activation ring: on
        # the Sync ring it queues behind the copies' descriptors and lands
        # ~12us late, delaying every gather.
        ib = const.tile([P, 2 * (JP // 16)], i16)
        nc.scalar.dma_start(ib[:, : JP // 16], ib_e[:][:, : JP // 16])
        fb = const.tile([P, FB_COLS], f32)
        nc.scalar.dma_start(fb[:], fb_e[:])
        nc.scalar.dma_start(ib[:, JP // 16 :], ib_e[:][:, JP // 16 :])
        wb = const.tile([P, HID + P], f16)
        nc.sync.dma_start(wb[:], wb_e[:])

        nreg = nc.gpsimd.to_reg(BU * P)  # one register for every num_idxs

        # Bulk copies immediately so the HW DMA engines start streaming.
        for u in range(U):
            nc.sync.dma_start(
                outs[u][0:ROWS_U, 0:S], x_e[u * ROWS_U : (u + 1) * ROWS_U, :]
            )
        # Mask halves on the Activation HWDGE queue.
        mask_g = big.tile([P, TP * S], f16)
        nc.scalar.dma_start(mask_g[:, : TP * S // 2], mask_e[:][:, : TP * S // 2])
        nc.scalar.dma_start(mask_g[:, TP * S // 2 :], mask_e[:][:, TP * S // 2 :])

        # Row-major gathers: one stream feeds both the MLP (via PE
        # transposes) and val.
        smp_gs = [
            big.tile([P, PIECES[gp] * SPH], f16, name=f"smp{gp}", tag=f"smp{gp}")
            for gp in range(NG)
        ]
        for gp in range(NG):
            jg = PIECES[gp] * P
            c0 = POFF[gp] * P // 16
            nc.gpsimd.dma_gather(
                out_ap=smp_gs[gp][:].rearrange("p (t e) -> p t e", e=SPH),
                in_ap=yq_e[:],
                idxs_ap=ib[:, c0 : c0 + jg // 16],
                num_idxs=jg,
                num_idxs_reg=nreg,
                elem_size=SPH,
                elem_step=SPH,
                single_packet=True,
                queue_num=GQ[gp],
            )

        ident = wb[:, HID : HID + P]
        ident4 = fb[0:4, 0:4]
        w2d_bd = fb[:, FB_W2D : FB_W2D + 4]

        # val tiles per scatter unit (precise dependencies).
        val_us = [
            big.tile([P, BU * S], f16, name=f"valu{u}", tag=f"valu{u}")
            for u in range(U)
        ]

        h4s = {}
        d4s_t = {}
        dts = {}
        smp3s = {}

        def stage_a(gp):
            """transposes -> DVE copy -> W1 matmul -> relus (d-fold bands)."""
            TGp = PIECES[gp]
            smp_g = smp_gs[gp]
            smp3s[gp] = smp_g[:].rearrange("p (t e) -> p t e", e=SPH)
            h4 = work.tile([P, 2 * P], f32, name=f"h4_{gp}", tag="h4")
            h4s[gp] = h4
            for b0 in range(0, TGp, 4):
                bl = min(4, TGp - b0)
                smp_tp = pp.tile(
                    [S + 2, 4 * P], f16, name=f"smp_tp{gp}_{b0}", tag="smp_tp"
                )
                for b in range(bl):
                    nc.tensor.transpose(
                        smp_tp[:, b * P : (b + 1) * P],
                        smp_g[:, (b0 + b) * SPH : (b0 + b) * SPH + S + 2],
                        ident,
                    )
                smp_ts = work.tile(
                    [S + 2, 4 * P], f16, name=f"smp_ts{gp}_{b0}", tag="smp_ts"
                )
                nc.vector.tensor_copy(
                    smp_ts[:, : bl * P], smp_tp[:, : bl * P]
                )
                hp = pph.tile([HID, 4 * P], f32, name=f"hp{gp}_{b0}", tag="hp")
                nc.tensor.matmul(
                    hp[:, : bl * P],
                    lhsT=wb[0 : S + 2, 0:HID],
                    rhs=smp_ts[:, : bl * P],
                    start=True,
                    stop=True,
                )
                for b in range(bl):
                    bb = b0 + b
                    g, j = bb // 4, bb % 4
                    nc.scalar.activation(
                        h4[HID * j : HID * (j + 1), g * P : (g + 1) * P],
                        hp[:, b * P : (b + 1) * P],
                        mybir.ActivationFunctionType.Relu,
                    )

        def stage_b(gp):
            """d4 -> d4s -> dt -> indicator -> val -> scatter."""
            TGp = PIECES[gp]
            t0 = POFF[gp]
            h4 = h4s[gp]
            G = (TGp + 3) // 4
            d4 = pp4.tile([4, 2 * P], f32, name=f"d4_{gp}", tag="d4")
            nfull = TGp // 4
            if nfull:
                nc.tensor.matmul(
                    d4[:, : nfull * P],
                    lhsT=w2d_bd,
                    rhs=h4[:, : nfull * P],
                    start=True,
                    stop=True,
                )
            rem = TGp - nfull * 4
            if rem:
                nc.tensor.matmul(
                    d4[0:rem, nfull * P : (nfull + 1) * P],
                    lhsT=w2d_bd[0 : HID * rem, 0:rem],
                    rhs=h4[0 : HID * rem, nfull * P : (nfull + 1) * P],
                    start=True,
                    stop=True,
                )
            d4s = work.tile([4, 2 * P], f32, name=f"d4s{gp}", tag="d4s")
            nc.scalar.activation(
                d4s[:, : G * P],
                d4[:, : G * P],
                mybir.ActivationFunctionType.Copy,
            )
            dt = ppt.tile([P, 8], f32, name=f"dt{gp}", tag="dt")
            for g in range(G):
                r = min(4, TGp - g * 4)
                nc.tensor.transpose(
                    dt[:, g * 4 : g * 4 + r],
                    d4s[0:r, g * P : (g + 1) * P],
                    ident4[0:r, 0:r],
                )
            ind = work.tile([P, TGp], f16, name=f"ind{gp}", tag="ind")
            nc.vector.tensor_tensor(
                out=ind[:],
                in0=dt[:, 0:TGp],
                in1=fb[:, FB_NE + t0 : FB_NE + t0 + TGp],
                op=mybir.AluOpType.is_gt,
            )
            u = gp
            vslice = val_us[u][:, 0 : TGp * S]
            v3 = vslice.rearrange("p (t s) -> p t s", s=S)
            nc.vector.tensor_tensor(
                out=v3,
                in0=smp3s[gp][:, :, 0:S],
                in1=mask_g[:, t0 * S : (t0 + TGp) * S].rearrange(
                    "p (t s) -> p t s", s=S
                ),
                op=mybir.AluOpType.mult,
            )
            ind_b = ind[:].unsqueeze(2).to_broadcast([P, TGp, S])
            nc.vector.tensor_tensor(
                out=v3, in0=v3, in1=ind_b, op=mybir.AluOpType.mult
            )
            # scatter-add this unit as soon as its val is ready; disjoint
            # 4004-row out tensors -> scatters overlap across queues
            jg = BU * P
            i0 = (JP + u * BU * P) // 16
            nc.gpsimd.dma_scatter_add(
                out_ap=outs[u][:][:, 0:S],
                in_ap=val_us[u][:].rearrange("p (t s) -> p t s", s=S),
                idxs_ap=ib[:, i0 : i0 + jg // 16],
                num_idxs=jg,
                num_idxs_reg=nreg,
                elem_size=S,
                elem_step=SPH,
                single_packet=True,
                queue_num=SQ[u],
            )

        # software pipeline: A(0) A(1) B(0) A(2) B(1) ... A(7) B(6) B(7)
        stage_a(0)
        for gp in range(1, NG):
            stage_a(gp)
            stage_b(gp - 1)
        stage_b(NG - 1)

    nc.compile()
    return nc


def _get_nc():
    if "nc" not in _cache:
        _cache["nc"] = _build()
    return _cache["nc"]


def _numpy_fallback(x, y, W1, b1, W2, b2, mask, gumbel, krig_idx, idx_of_node):
    offs = np.concatenate([[0], np.cumsum(idx_of_node.astype(np.int64))[:-1]])
    flat = (offs[:, None] + krig_idx).reshape(-1)
    smp = y[flat]
    h = np.maximum(smp.astype(np.float32) @ W1 + b1, 0.0)
    logits = h @ W2 + b2
    z = logits + gumbel
    ind = (z[:, 1] > z[:, 0]).astype(np.float32)
    val = ind[:, None] * mask * smp
    out = x.copy()
    out[flat] = val
    return out


def kernel(**inputs) -> np.ndarray:
    x = np.ascontiguousarray(inputs["x"], dtype=np.float32)
    y = np.ascontiguousarray(inputs["y"], dtype=np.float32)
    W1 = np.ascontiguousarray(inputs["W1"], dtype=np.float32)
    b1 = np.ascontiguousarray(inputs["b1"], dtype=np.float32)
    W2 = np.ascontiguousarray(inputs["W2"], dtype=np.float32)
    b2 = np.ascontiguousarray(inputs["b2"], dtype=np.float32)
    mask = np.ascontiguousarray(inputs["mask"], dtype=np.float32)
    gumbel = np.ascontiguousarray(inputs["gumbel"], dtype=np.float32)
    krig = np.asarray(inputs["krig_idx"]).astype(np.int64)
    ion = np.asarray(inputs["idx_of_node"]).astype(np.int64)

    if (
        x.shape != (BS * N, S)
        or krig.shape != (BS, K)
        or not np.all(ion == N)
        or krig.min() < 0
        or krig.max() >= N
    ):
        return _numpy_fallback(
            x, y, W1, b1, W2, b2, mask, gumbel,
            np.asarray(inputs["krig_idx"]), ion,
        )

    from concourse.bass_utils import run_bass_kernel_spmd

    nc = _get_nc()

    # Host layout prep (sharding/marshalling only).
    flat_all = ((np.arange(BS, dtype=np.int64) * N)[:, None] + krig).reshape(-1)
    xz = x.copy()
    xz[flat_all] = 0.0  # scatter targets become add-onto-zero
    xh = xz.astype(np.float16)
    yq = np.zeros((M, R, SPH), dtype=np.float16)
    yq[:, :, :S] = y.reshape(M, R, S).astype(np.float16)
    yq[:, :, S] = 1.0  # constant-1 column folds b1 into the W1 matmul
    # padded token stream: per unit, 800 real tokens then 96 dummies
    pad_sel = np.zeros(JP, dtype=np.int64)
    pad_real = np.zeros(JP, dtype=bool)
    for u in range(U):
        pad_sel[u * BU * P : u * BU * P + JU] = np.arange(JU) + u * JU
        pad_real[u * BU * P : u * BU * P + JU] = True

    gd = (gumbel[:, 1] - gumbel[:, 0] + (b2[1] - b2[0])).astype(np.float32)
    w2d = (W2[:, 1] - W2[:, 0]).astype(np.float32)

    def wrap16(stream):
        # device consumes index i at idxs[i % 16, i // 16], replicated x8
        return np.ascontiguousarray(
            np.tile(stream.reshape(-1, 16).T.astype(np.int16), (M, 1))
        )

    fb_common = np.zeros((P, FB_COLS), dtype=np.float32)
    fb_common[0:4, 0:4] = np.eye(4, dtype=np.float32)
    for j in range(4):
        fb_common[HID * j : HID * (j + 1), FB_W2D + j] = w2d
    wb = np.zeros((P, HID + P), dtype=np.float16)
    wb[:S, :HID] = W1.astype(np.float16)
    wb[S, :HID] = b1.astype(np.float16)
    wb[:, HID:] = np.eye(P, dtype=np.float16)

    mask16 = mask.astype(np.float16)
    in_maps = []
    for m in range(M):
        fl = flat_all[m * J : (m + 1) * J] - m * R   # [J] core-local rows
        flp = fl[pad_sel]                            # [JP] padded stream
        sidx = flp - (np.arange(JP) // (BU * P)) * ROWS_U
        # pads add zero-vals to sacrificial rows (real rows would race:
        # concurrent CCE read-modify-writes on one row lose updates)
        sidx[~pad_real] = ROWS_U + (np.arange((~pad_real).sum()) % 4)
        ib = np.concatenate([wrap16(flp), wrap16(sidx)], axis=1)
        mk = mask16[m * J : (m + 1) * J][pad_sel]    # [JP, S]
        mk[~pad_real] = 0
        mask_r = np.ascontiguousarray(
            mk.reshape(TP, P, S).transpose(1, 0, 2)
        ).reshape(P, TP * S)
        ne = -gd[m * J : (m + 1) * J][pad_sel]
        ne[~pad_real] = 1e9                          # pad ind -> 0
        fb = fb_common.copy()
        fb[:, FB_NE : FB_NE + TP] = np.ascontiguousarray(
            ne.reshape(TP, P).T
        )
        in_maps.append(
            {
                "x": xh[m * R : (m + 1) * R],
                "yq": yq[m],
                "mask": mask_r,
                "ib": ib,
                "fb": fb,
                "wb": wb,
            }
        )

    import os

    trace = bool(int(os.environ.get("KERNEL_TRACE", "0")))
    res = run_bass_kernel_spmd(nc, in_maps, core_ids=list(range(M)), trace=trace)
    _cache["last_res"] = res

    out = np.empty((BS * N, S), dtype=np.float32)
    for m in range(M):
        for u in range(U):
            out[m * R + u * ROWS_U : m * R + (u + 1) * ROWS_U] = (
                np.asarray(res.results[m][f"out{u}"])[:ROWS_U, :S].astype(np.float32)
            )
    return out
